# revision 1
# baseline (speedup 1.0000x reference)
"""Trainium2 kernel for nn_IteratedLinearNet: y = x @ (W.T)^60.

Strategy (8 NeuronCores, single SPMD launch):
  - matrix power by squaring via the addition chain 2, 4, 8, 12, 24, 48, 60
    (7 matmuls of 2048^3 instead of 60 applications of x @ W.T)
  - each product is tensor-sharded: core j computes a 256-wide column slab
  - after each product (except the last) the core transposes its slab on
    TensorE and an 8-core AllGather assembles the full transposed matrix,
    which is the next product's stationary operand; AllGathers are split
    into column halves so compute pipelines with communication
  - final apply is tensor-parallel: core j computes y[:, Sj] for the full
    batch with x.T streamed from HBM
  - all matmuls run in float32r (FP22-truncated reads, full PE rate);
    inputs are pre-rounded to FP22-nearest on the host to keep the
    truncation exact and unbiased

Self-contained: builds/compiles on first call and caches the module.
"""

import numpy as np

_GRID = 2048
_BATCH = 4096
_NCORES = 8
_SW = _GRID // _NCORES  # 256
_KT = _GRID // 128  # 16
_HALF = _GRID // 2

# (power, lhsT_src, rhs_buf, out_buf); lhsT_src: "wt" or index of the step
# whose AllGather output (the transposed full matrix) is the stationary side.
_CHAIN = [
    (2, "wt", 0, 1),
    (4, 0, 1, 2),
    (8, 1, 2, 0),
    (12, 2, 2, 0),  # A12 = A8 @ A4 (rhs = A4 slab, still in buf 2)
    (24, 3, 0, 1),
    (48, 4, 1, 2),
    (60, 5, 0, 1),
]

_cache = {}


def _build():
    from contextlib import ExitStack

    import concourse.tile as tile
    from concourse import bacc, masks, mybir

    F32R = mybir.dt.float32r
    F32 = mybir.dt.float32
    G, KT, SW, HALF, BATCH = _GRID, _KT, _SW, _HALF, _BATCH

    nc = bacc.Bacc(None, target_bir_lowering=False, num_devices=_NCORES)
    wt = nc.declare_dram_parameter("wt", [G, G], F32R, isOutput=False)
    aslab = nc.declare_dram_parameter("aslab", [G, SW], F32R, isOutput=False)
    xt = nc.declare_dram_parameter("xt", [G, BATCH], F32R, isOutput=False)
    ytj = nc.declare_dram_parameter("ytj", [SW, BATCH], F32R, isOutput=True)

    rg = [list(range(_NCORES))]

    with ExitStack() as ctx:
        tc = ctx.enter_context(tile.TileContext(nc))
        big = ctx.enter_context(tc.tile_pool(name="big", bufs=1))
        slabs = ctx.enter_context(tc.tile_pool(name="slabs", bufs=1))
        shpool = ctx.enter_context(tc.tile_pool(name="shpool", bufs=3))
        ypool = ctx.enter_context(tc.tile_pool(name="ypool", bufs=2))
        mmps = ctx.enter_context(tc.tile_pool(name="mmps", bufs=4, space="PSUM"))
        tps = ctx.enter_context(tc.tile_pool(name="tps", bufs=2, space="PSUM"))
        dram = ctx.enter_context(tc.tile_pool(name="dram", bufs=2, space="DRAM"))

        lhsT_sb = big.tile([128, KT, G], F32R)
        sbuf = [
            slabs.tile([128, KT, SW], F32R, name=f"slab{i}", tag=f"slab{i}")
            for i in range(3)
        ]
        ident32 = slabs.tile([128, 128], F32, name="ident32", tag="ident32")
        masks.make_identity(nc, ident32[:])
        ident = slabs.tile([128, 128], F32R, name="ident", tag="ident")
        nc.vector.tensor_copy(ident[:], ident32[:])

        for k in range(KT):
            nc.sync.dma_start(sbuf[0][:, k, :], aslab[128 * k : 128 * (k + 1), :])

        ag_outs = []
        n_steps = len(_CHAIN)
        for si, (power, src, rb, ob) in enumerate(_CHAIN):
            is_last = si == n_steps - 1
            rhs = sbuf[rb]
            out = sbuf[ob]
            ag_out_halves = []
            for h in range(2):
                for k in range(KT):
                    if src == "wt":
                        s_ap = wt[128 * k : 128 * (k + 1), HALF * h : HALF * (h + 1)]
                    else:
                        s_ap = ag_outs[src][h][128 * k : 128 * (k + 1), :]
                    nc.sync.dma_start(lhsT_sb[:, k, HALF * h : HALF * (h + 1)], s_ap)
                for m in range(8 * h, 8 * h + 8):
                    ps = mmps.tile([128, SW], F32, name="ps", tag="ps")
                    for k in range(KT):
                        nc.tensor.matmul(
                            ps[:],
                            lhsT_sb[:, k, 128 * m : 128 * (m + 1)],
                            rhs[:, k, :],
                            start=(k == 0),
                            stop=(k == KT - 1),
                        )
                    nc.vector.tensor_copy(out[:, m, :], ps[:])
                if is_last:
                    continue
                t_sb = shpool.tile([128, 2, HALF], F32R, name=f"t{si}_{h}", tag="sh8")
                for k in range(8 * h, 8 * h + 8):
                    for a in range(2):
                        psT = tps.tile([128, 128], F32R, name="psT", tag="psT")
                        nc.tensor.transpose(
                            psT[:], out[:, k, 128 * a : 128 * (a + 1)], ident[:]
                        )
                        nc.vector.tensor_copy(
                            t_sb[:, a, 128 * (k - 8 * h) : 128 * (k - 8 * h + 1)],
                            psT[:],
                        )
                ag_in = dram.tile([SW, HALF], F32R, name=f"agin{si}_{h}", tag="agin")
                for a in range(2):
                    nc.sync.dma_start(ag_in[128 * a : 128 * (a + 1), :], t_sb[:, a, :])
                ag_out = dram.tile(
                    [G, HALF],
                    F32R,
                    name=f"agout{si}_{h}",
                    tag="agout",
                    addr_space="Shared",
                )
                nc.gpsimd.collective_compute(
                    "AllGather",
                    mybir.AluOpType.bypass,
                    replica_groups=rg,
                    ins=[ag_in.opt()],
                    outs=[ag_out.opt()],
                )
                ag_out_halves.append(ag_out)
            ag_outs.append(ag_out_halves)

        final = sbuf[_CHAIN[-1][3]]
        for c in range(BATCH // SW):
            pss = [
                mmps.tile([128, SW], F32, name=f"psy{a}", tag="ps") for a in range(2)
            ]
            for kh in range(2):
                xchunk = shpool.tile([128, KT // 2, SW], F32R, name="xchunk", tag="sh8")
                for kk in range(KT // 2):
                    k = 8 * kh + kk
                    nc.sync.dma_start(
                        xchunk[:, kk, :],
                        xt[128 * k : 128 * (k + 1), SW * c : SW * (c + 1)],
                    )
                for a in range(2):
                    for kk in range(KT // 2):
                        k = 8 * kh + kk
                        nc.tensor.matmul(
                            pss[a][:],
                            final[:, k, 128 * a : 128 * (a + 1)],
                            xchunk[:, kk, :],
                            start=(k == 0),
                            stop=(k == KT - 1),
                        )
            for a in range(2):
                ystage = ypool.tile([128, SW], F32R, name="ystage", tag="ystage")
                nc.vector.tensor_copy(ystage[:], pss[a][:])
                nc.sync.dma_start(
                    ytj[128 * a : 128 * (a + 1), SW * c : SW * (c + 1)], ystage[:]
                )
    nc.compile()
    return nc


def _round22(a):
    bits = np.ascontiguousarray(a).view(np.uint32)
    return ((bits + 0x200) & np.uint32(0xFFFFFC00)).view(np.float32)


def kernel(x, W):
    from concourse.bass_utils import run_bass_kernel_spmd

    if "nc" not in _cache:
        _cache["nc"] = _build()
    nc = _cache["nc"]

    Wr = _round22(np.asarray(W, dtype=np.float32))
    xr = _round22(np.asarray(x, dtype=np.float32))
    wt_np = np.ascontiguousarray(Wr)
    xt_np = np.ascontiguousarray(xr.T)
    in_maps = [
        {
            "wt": wt_np,
            "aslab": np.ascontiguousarray(Wr[_SW * j : _SW * (j + 1), :].T),
            "xt": xt_np,
        }
        for j in range(_NCORES)
    ]
    res = run_bass_kernel_spmd(nc, in_maps, core_ids=list(range(_NCORES)))
    _cache["last_exec_time_ns"] = res.exec_time_ns
    _cache["last_results"] = res
    y = np.concatenate(
        [res.results[j]["ytj"].T for j in range(_NCORES)], axis=1
    ).astype(np.float32)
    return y



# revision 5
# speedup vs baseline: 1.3054x; 1.3054x over previous
"""Trainium2 kernel for nn_IteratedLinearNet: y = x @ (W.T)^60.

Strategy (8 NeuronCores, single SPMD launch):
  - track T_k = W^k (so y.T = T60-scaled @ x.T); matrix power by the
    addition chain 2, 4, 8, 12=8+4, 24, 48, 60=12+48 (7 products + 1
    batch-parallel apply)
  - row-sharded slabs: core j holds U_k = (T_k[Rj,:]).T  [2048, 256];
    each product is U_next = G.T @ U with the full gathered G = T_a
    native as the stationary operand (lhsT = G directly)
  - slabs are AllGathered raw (no on-chip transpose at all); the next
    stationary is loaded with XBAR DMA-transpose (2-byte dtype) straight
    from the gathered buffer, at normal DMA throughput
  - T8 is only ever a local rhs (never a stationary), so step 3 has no
    AllGather, and steps 3+4 share the T4 stationary
  - everything in float16 at full PE rate with per-step power-of-2
    rescaling (exact, folded into the PSUM evacuation) to keep values
    in fp16 range; PSUM accumulates fp32; host multiplies the fp32
    output by 2^-52.  Simulated end-to-end rel err ~1.8e-3 (gate 2e-2).
  - final apply: core j computes y[Bj,:].T for its 512 batch rows; the
    raw U60 AllGather block layout is directly the A60 stationary

Self-contained: builds/compiles on first call and caches the module.
"""

import numpy as np

_G = 2048
_B = 4096
_NC = 8
_SW = _G // _NC  # 256 slab width (rows per core)
_BW = _B // _NC  # 512 batch rows per core
_KT = _G // 128  # 16
_HALF = _G // 2

# (out_power, rhs_slab, out_slab, scale_shift, do_ag)
# slabs rotate A,B,C; B holds U12 from step 4 until step 7.
_STEPS = [
    (2, 0, 1, 6, True),
    (4, 1, 2, -4, True),
    (8, 2, 0, -5, False),  # T8 only ever a local rhs
    (12, 0, 1, -5, True),  # same stationary as step 3 (T4)
    (24, 1, 2, -5, True),
    (48, 2, 0, -4, True),
    (60, 1, 2, -4, None),  # rhs = U12 (slab B); raw-slab AG for apply
]
_TOTAL_SHIFT = 52

_cache = {}


def _build():
    from contextlib import ExitStack

    import concourse.tile as tile
    from concourse import bacc, mybir

    F16 = mybir.dt.float16
    F32 = mybir.dt.float32
    G, KT, SW, BW, HALF = _G, _KT, _SW, _BW, _HALF

    nc = bacc.Bacc(None, target_bir_lowering=False, num_devices=_NC)
    w16 = nc.declare_dram_parameter("w16", [G, G], F16, isOutput=False)
    u1 = nc.declare_dram_parameter("u1", [G, SW], F16, isOutput=False)
    xt = nc.declare_dram_parameter("xt", [G, BW], F16, isOutput=False)
    ytj = nc.declare_dram_parameter("ytj", [G, BW], F32, isOutput=True)

    rg = [list(range(_NC))]

    with ExitStack() as ctx:
        tc = ctx.enter_context(tile.TileContext(nc))
        statp = ctx.enter_context(tc.tile_pool(name="statp", bufs=1))
        slabs = ctx.enter_context(tc.tile_pool(name="slabs", bufs=1))
        ypool = ctx.enter_context(tc.tile_pool(name="ypool", bufs=2))
        mmps = ctx.enter_context(tc.tile_pool(name="mmps", bufs=4, space="PSUM"))
        yps = ctx.enter_context(tc.tile_pool(name="yps", bufs=2, space="PSUM"))
        dram = ctx.enter_context(tc.tile_pool(name="dram", bufs=1, space="DRAM"))

        stat = [
            statp.tile([128, KT, G], F16, name=f"stat{i}", tag=f"stat{i}")
            for i in range(2)
        ]
        slab = [
            slabs.tile([128, KT, SW], F16, name=f"slab{i}", tag=f"slab{i}")
            for i in range(3)
        ]
        xts = slabs.tile([128, KT, BW], F16, name="xts", tag="xts")

        # prologue: first slab + first stationary (W itself)
        for k in range(KT):
            nc.sync.dma_start(slab[0][:, k, :], u1[128 * k : 128 * (k + 1), :])
        for k in range(KT):
            nc.sync.dma_start(stat[0][:, k, :], w16[128 * k : 128 * (k + 1), :])

        sb_idx = 0  # which stat buffer the current step reads
        for si, (pw, rb, ob, shift, do_ag) in enumerate(_STEPS):
            st = stat[sb_idx]
            rhs = slab[rb]
            out = slab[ob]
            scale = float(2.0**shift)
            last = do_ag is None
            for h in range(2):
                for m in range(8 * h, 8 * h + 8):
                    ps = mmps.tile([128, SW], F32, name="ps", tag="ps")
                    for k in range(KT):
                        nc.tensor.matmul(
                            ps[:],
                            st[:, k, 128 * m : 128 * (m + 1)],
                            rhs[:, k, :],
                            start=(k == 0),
                            stop=(k == KT - 1),
                        )
                    nc.vector.tensor_scalar_mul(out[:, m, :], ps[:], scale)
                if last or not do_ag:
                    continue
                # ship this half of the raw slab; gathered blocks are
                # transposed into the next stationary by the XBAR DMA
                u_in = dram.tile(
                    [HALF, SW], F16, name=f"uin{pw}_{h}", tag=f"uin{pw}_{h}"
                )
                for m in range(8 * h, 8 * h + 8):
                    mr = m - 8 * h
                    nc.sync.dma_start(
                        u_in[128 * mr : 128 * (mr + 1), :], out[:, m, :]
                    )
                g = dram.tile(
                    [_NC * HALF, SW],
                    F16,
                    name=f"g{pw}_{h}",
                    tag=f"g{pw}_{h}",
                    addr_space="Shared",
                )
                nc.gpsimd.collective_compute(
                    "AllGather",
                    mybir.AluOpType.bypass,
                    replica_groups=rg,
                    ins=[u_in.opt()],
                    outs=[g.opt()],
                )
                nst = stat[1 - sb_idx]
                for kt in range(KT):
                    j = kt // 2
                    c0 = 128 * (kt % 2)
                    nc.sync.dma_start(
                        nst[:, kt, HALF * h : HALF * (h + 1)],
                        g[HALF * j : HALF * (j + 1), c0 : c0 + 128],
                        transpose=True,
                    )
            if do_ag:
                sb_idx = 1 - sb_idx
            # step 3 (T8): no AG; step 4 reuses the same stationary (T4)

        # stream x.T in (needed only for the apply)
        for k in range(KT):
            nc.sync.dma_start(xts[:, k, :], xt[128 * k : 128 * (k + 1), :])

        # final raw-slab AllGather: g60 blocks are directly A60-scaled
        u60 = slab[_STEPS[-1][2]]
        u60_in = dram.tile([G, SW], F16, name="u60in", tag="u60in")
        for k in range(KT):
            nc.sync.dma_start(u60_in[128 * k : 128 * (k + 1), :], u60[:, k, :])
        g60 = dram.tile(
            [G * _NC, SW], F16, name="g60", tag="g60", addr_space="Shared"
        )
        nc.gpsimd.collective_compute(
            "AllGather",
            mybir.AluOpType.bypass,
            replica_groups=rg,
            ins=[u60_in.opt()],
            outs=[g60.opt()],
        )

        # apply: ytj = (x_j @ A60s).T ; A60s[f, f'] = g60[G*(f'//SW) + f, f'%SW]
        ast = stat[1 - sb_idx]
        for j in range(_NC):
            for k in range(KT):
                nc.sync.dma_start(
                    ast[:, k, SW * j : SW * (j + 1)],
                    g60[G * j + 128 * k : G * j + 128 * (k + 1), :],
                )
        for m in range(KT):
            psY = yps.tile([128, BW], F32, name="psY", tag="psY")
            for k in range(KT):
                nc.tensor.matmul(
                    psY[:],
                    ast[:, k, 128 * m : 128 * (m + 1)],
                    xts[:, k, :],
                    start=(k == 0),
                    stop=(k == KT - 1),
                )
            ystage = ypool.tile([128, BW], F32, name="ystage", tag="ystage")
            nc.vector.tensor_copy(ystage[:], psY[:])
            nc.sync.dma_start(ytj[128 * m : 128 * (m + 1), :], ystage[:])
    nc.compile()
    return nc


def kernel(x, W):
    from concourse.bass_utils import run_bass_kernel_spmd

    if "nc" not in _cache:
        _cache["nc"] = _build()
    nc = _cache["nc"]

    W = np.asarray(W, dtype=np.float32)
    x = np.asarray(x, dtype=np.float32)
    w16_np = np.ascontiguousarray(W.astype(np.float16))
    xt_np = x.T.astype(np.float16)
    in_maps = [
        {
            "w16": w16_np,
            "u1": np.ascontiguousarray(w16_np[_SW * j : _SW * (j + 1), :].T),
            "xt": np.ascontiguousarray(xt_np[:, _BW * j : _BW * (j + 1)]),
        }
        for j in range(_NC)
    ]
    res = run_bass_kernel_spmd(nc, in_maps, core_ids=list(range(_NC)))
    _cache["last_exec_time_ns"] = res.exec_time_ns
    _cache["last_results"] = res
    unscale = np.float32(2.0**-_TOTAL_SHIFT)
    y = np.concatenate(
        [res.results[j]["ytj"].T * unscale for j in range(_NC)], axis=0
    ).astype(np.float32)
    return y


# revision 7
# speedup vs baseline: 1.6423x; 1.2581x over previous
"""Trainium2 kernel for nn_IteratedLinearNet: y = x @ (W.T)^60.

Strategy (8 NeuronCores, single SPMD launch):
  - track T_k = W^k; build T60 with 9 products using only THREE
    stationary matrices {W, T3, T12}:
      T2=T1*W, T3=T2*W            (stationary: W, from DRAM input)
      T6=T3*T3, T9=T6*T3, T12=T9*T3   (stationary: gathered T3)
      T24=T12*T12, T36, T48, T60      (stationary: gathered T12)
    so only 3 AllGathers total (T3, T12, T60) — collectives cost
    ~25-40us each on this fabric and dominate if done per step
  - row-sharded slabs: core j holds U_k = (T_k[Rj,:]).T [2048, 256];
    each product is U_next = G.T @ U with gathered G native as lhsT;
    every step's rhs is simply the previous step's output (3-slab ring)
  - slabs are AllGathered raw; the next stationary is produced by XBAR
    DMA-transpose (2-byte dtype) straight from the gathered buffer;
    mid-chain AllGathers are split in column halves so the consumer's
    first half-step overlaps the second half's gather
  - all DMAs alternate between the two hardware DGE queues (SP + Act)
  - float16 at full PE rate with per-step power-of-2 rescaling (exact,
    folded into PSUM evacuation); PSUM accumulates fp32; host multiplies
    the fp32 output by 2^-52.  Simulated end-to-end rel err ~2.3e-3.
  - final apply: core j computes y[Bj,:].T for its 512 batch rows; the
    raw U60 AllGather block layout is directly the A60 stationary

Self-contained: builds/compiles on first call and caches the module.
"""

import numpy as np

_G = 2048
_B = 4096
_NC = 8
_SW = _G // _NC  # 256 slab width (rows per core)
_BW = _B // _NC  # 512 batch rows per core
_KT = _G // 128  # 16
_HALF = _G // 2

# (power, stat: 0=W/g12(stat0) 1=g3(stat1), shift, export: None|"ag")
# slab ring: step i reads slab[i%3], writes slab[(i+1)%3]
_STEPS = [
    (2, 0, 6, None),
    (3, 0, 1, "ag"),  # -> g3 (stationary for T6/T9/T12)
    (6, 1, -5, None),
    (9, 1, -4, None),
    (12, 1, -5, "ag"),  # -> g12 (stationary for T24..T60)
    (24, 0, -5, None),
    (36, 0, -4, None),
    (48, 0, -5, None),
    (60, 0, -4, "final"),
]
_TOTAL_SHIFT = 52

_cache = {}


def _build():
    from contextlib import ExitStack

    import concourse.tile as tile
    from concourse import bacc, mybir

    F16 = mybir.dt.float16
    F32 = mybir.dt.float32
    G, KT, SW, BW, HALF = _G, _KT, _SW, _BW, _HALF

    nc = bacc.Bacc(None, target_bir_lowering=False, num_devices=_NC)
    w16 = nc.declare_dram_parameter("w16", [G, G], F16, isOutput=False)
    u1 = nc.declare_dram_parameter("u1", [G, SW], F16, isOutput=False)
    xt = nc.declare_dram_parameter("xt", [G, BW], F16, isOutput=False)
    ytj = nc.declare_dram_parameter("ytj", [G, BW], F32, isOutput=True)

    rg = [list(range(_NC))]

    with ExitStack() as ctx:
        tc = ctx.enter_context(tile.TileContext(nc))
        statp = ctx.enter_context(tc.tile_pool(name="statp", bufs=1))
        slabs = ctx.enter_context(tc.tile_pool(name="slabs", bufs=1))
        ypool = ctx.enter_context(tc.tile_pool(name="ypool", bufs=2))
        mmps = ctx.enter_context(tc.tile_pool(name="mmps", bufs=4, space="PSUM"))
        yps = ctx.enter_context(tc.tile_pool(name="yps", bufs=2, space="PSUM"))
        dram = ctx.enter_context(tc.tile_pool(name="dram", bufs=1, space="DRAM"))

        stat = [
            statp.tile([128, KT, G], F16, name=f"stat{i}", tag=f"stat{i}")
            for i in range(2)
        ]
        slab = [
            slabs.tile([128, KT, SW], F16, name=f"slab{i}", tag=f"slab{i}")
            for i in range(3)
        ]
        xts = slabs.tile([128, KT, BW], F16, name="xts", tag="xts")

        q = [nc.sync, nc.sync]  # BISECT: all DMAs on SP queue

        # prologue: first slab + first stationary (W itself)
        for k in range(KT):
            q[k % 2].dma_start(slab[0][:, k, :], u1[128 * k : 128 * (k + 1), :])
        for k in range(KT):
            q[k % 2].dma_start(stat[0][:, k, :], w16[128 * k : 128 * (k + 1), :])

        for si, (pw, sb, shift, export) in enumerate(_STEPS):
            st = stat[sb]
            rhs = slab[si % 3]
            out = slab[(si + 1) % 3]
            scale = float(2.0**shift)
            for h in range(2):
                for m in range(8 * h, 8 * h + 8):
                    ps = mmps.tile([128, SW], F32, name="ps", tag="ps")
                    for k in range(KT):
                        nc.tensor.matmul(
                            ps[:],
                            st[:, k, 128 * m : 128 * (m + 1)],
                            rhs[:, k, :],
                            start=(k == 0),
                            stop=(k == KT - 1),
                        )
                    nc.vector.tensor_scalar_mul(out[:, m, :], ps[:], scale)
                if export != "ag":
                    continue
                # mid-chain AllGather, split in column halves for overlap
                u_in = dram.tile(
                    [HALF, SW], F16, name=f"uin{pw}_{h}", tag=f"uin{pw}_{h}"
                )
                for m in range(8 * h, 8 * h + 8):
                    mr = m - 8 * h
                    q[m % 2].dma_start(
                        u_in[128 * mr : 128 * (mr + 1), :], out[:, m, :]
                    )
                g = dram.tile(
                    [_NC * HALF, SW],
                    F16,
                    name=f"g{pw}_{h}",
                    tag=f"g{pw}_{h}",
                    addr_space="Shared",
                )
                nc.gpsimd.collective_compute(
                    "AllGather",
                    mybir.AluOpType.bypass,
                    replica_groups=rg,
                    ins=[u_in.opt()],
                    outs=[g.opt()],
                )
                # XBAR-transpose the gathered raw slabs into the next
                # stationary: stat[:, k, m(half h)] = T[128k+p, m]
                nst = stat[1 - sb]
                for k in range(KT):
                    j = k // 2
                    c0 = 128 * (k % 2)
                    q[k % 2].dma_start(
                        nst[:, k, HALF * h : HALF * (h + 1)],
                        g[HALF * j : HALF * (j + 1), c0 : c0 + 128],
                        transpose=True,
                    )
            if si == 1:
                # stream x.T during the long T6/T9/T12 burst
                for k in range(KT):
                    q[k % 2].dma_start(
                        xts[:, k, :], xt[128 * k : 128 * (k + 1), :]
                    )

        # final raw-slab AllGather: g60 blocks are directly A60-scaled
        u60 = slab[len(_STEPS) % 3]
        u60_in = dram.tile([G, SW], F16, name="u60in", tag="u60in")
        for k in range(KT):
            q[k % 2].dma_start(u60_in[128 * k : 128 * (k + 1), :], u60[:, k, :])
        g60 = dram.tile(
            [G * _NC, SW], F16, name="g60", tag="g60", addr_space="Shared"
        )
        nc.gpsimd.collective_compute(
            "AllGather",
            mybir.AluOpType.bypass,
            replica_groups=rg,
            ins=[u60_in.opt()],
            outs=[g60.opt()],
        )

        # apply: ytj = (x_j @ A60s).T ; A60s[f, f'] = g60[G*(f'//SW) + f, f'%SW]
        ast = stat[1]
        for j in range(_NC):
            for k in range(KT):
                q[k % 2].dma_start(
                    ast[:, k, SW * j : SW * (j + 1)],
                    g60[G * j + 128 * k : G * j + 128 * (k + 1), :],
                )
        for m in range(KT):
            psY = yps.tile([128, BW], F32, name="psY", tag="psY")
            for k in range(KT):
                nc.tensor.matmul(
                    psY[:],
                    ast[:, k, 128 * m : 128 * (m + 1)],
                    xts[:, k, :],
                    start=(k == 0),
                    stop=(k == KT - 1),
                )
            ystage = ypool.tile([128, BW], F32, name="ystage", tag="ystage")
            nc.vector.tensor_copy(ystage[:], psY[:])
            q[m % 2].dma_start(ytj[128 * m : 128 * (m + 1), :], ystage[:])
    nc.compile()
    return nc


def kernel(x, W):
    from concourse.bass_utils import run_bass_kernel_spmd

    if "nc" not in _cache:
        _cache["nc"] = _build()
    nc = _cache["nc"]

    W = np.asarray(W, dtype=np.float32)
    x = np.asarray(x, dtype=np.float32)
    w16_np = np.ascontiguousarray(W.astype(np.float16))
    xt_np = x.T.astype(np.float16)
    in_maps = [
        {
            "w16": w16_np,
            "u1": np.ascontiguousarray(w16_np[_SW * j : _SW * (j + 1), :].T),
            "xt": np.ascontiguousarray(xt_np[:, _BW * j : _BW * (j + 1)]),
        }
        for j in range(_NC)
    ]
    res = run_bass_kernel_spmd(nc, in_maps, core_ids=list(range(_NC)))
    _cache["last_exec_time_ns"] = res.exec_time_ns
    _cache["last_results"] = res
    unscale = np.float32(2.0**-_TOTAL_SHIFT)
    y = np.concatenate(
        [res.results[j]["ytj"].T * unscale for j in range(_NC)], axis=0
    ).astype(np.float32)
    return y


# revision 10
# speedup vs baseline: 1.6903x; 1.0292x over previous
"""Trainium2 kernel for nn_IteratedLinearNet: y = x @ (W.T)^60.

Strategy (8 NeuronCores, single SPMD launch):
  - track T_k = W^k; build T60 with 9 products using only THREE
    stationary matrices {W, T3, T12}:
      T2=T1*W, T3=T2*W            (stationary: W, from DRAM input)
      T6=T3*T3, T9=T6*T3, T12=T9*T3   (stationary: gathered T3)
      T24=T12*T12, T36, T48, T60      (stationary: gathered T12)
    so only 3 AllGathers total (T3, T12, T60) — collectives cost
    ~25-40us each on this fabric and dominate if done per step
  - row-sharded slabs: core j holds U_k = (T_k[Rj,:]).T [2048, 256];
    each product is U_next = G.T @ U with gathered G native as lhsT;
    every step's rhs is simply the previous step's output (3-slab ring)
  - slabs are AllGathered raw (one AG per point — split halves just
    serialize on the collective stream here); the next stationary is
    produced by XBAR DMA-transpose from the gathered buffer, one
    [2048,128] -> [128,2048] row per k-tile
  - steps that follow a stationary load run their matmuls K-MAJOR with
    8 open PSUM groups, so MM consumption (8x107ns per row) tracks the
    ~0.9us/row stationary delivery instead of waiting for all 16 rows
  - plain DMAs alternate between the SP HWDGE queue and the gpsimd
    software DGE (the Activation HWDGE queue corrupts data on this
    runtime); XBAR transposes are HWDGE-only so they stay on SP
  - float16 at full PE rate with per-step power-of-2 rescaling (exact,
    folded into PSUM evacuation); PSUM accumulates fp32; host multiplies
    the fp32 output by 2^-52.  Simulated end-to-end rel err ~2.3e-3.
  - final apply: core j computes y[Bj,:].T for its 512 batch rows; the
    raw U60 AllGather block layout is directly the A60 stationary,
    chunk-loaded j-block-major so the 2 MM tiles per block chase the DMA

Self-contained: builds/compiles on first call and caches the module.
"""

import numpy as np

_G = 2048
_B = 4096
_NC = 8
_SW = _G // _NC  # 256 slab width (rows per core)
_BW = _B // _NC  # 512 batch rows per core
_KT = _G // 128  # 16
_HALF = _G // 2

# (power, stat: 0=W/g12(stat0) 1=g3(stat1), shift, export, kmajor)
# slab ring: step i reads slab[i%3], writes slab[(i+1)%3]
_STEPS = [
    (2, 0, 6, False, True),
    (3, 0, 1, True, False),  # -> g3 (stationary for T6/T9/T12)
    (6, 1, -5, False, True),
    (9, 1, -4, False, False),
    (12, 1, -5, True, False),  # -> g12 (stationary for T24..T60)
    (24, 0, -5, False, True),
    (36, 0, -4, False, False),
    (48, 0, -5, False, False),
    (60, 0, -4, False, False),
]
_TOTAL_SHIFT = 52

_cache = {}


def _build():
    from contextlib import ExitStack

    import concourse.tile as tile
    from concourse import bacc, mybir

    F16 = mybir.dt.float16
    F32 = mybir.dt.float32
    G, KT, SW, BW, HALF = _G, _KT, _SW, _BW, _HALF

    nc = bacc.Bacc(None, target_bir_lowering=False, num_devices=_NC)
    w16 = nc.declare_dram_parameter("w16", [G, G], F16, isOutput=False)
    u1 = nc.declare_dram_parameter("u1", [G, SW], F16, isOutput=False)
    xt = nc.declare_dram_parameter("xt", [G, BW], F16, isOutput=False)
    ytj = nc.declare_dram_parameter("ytj", [G, BW], F32, isOutput=True)

    rg = [list(range(_NC))]

    with ExitStack() as ctx:
        tc = ctx.enter_context(tile.TileContext(nc))
        statp = ctx.enter_context(tc.tile_pool(name="statp", bufs=1))
        slabs = ctx.enter_context(tc.tile_pool(name="slabs", bufs=1))
        ypool = ctx.enter_context(tc.tile_pool(name="ypool", bufs=2))
        mmps = ctx.enter_context(tc.tile_pool(name="mmps", bufs=8, space="PSUM"))
        dram = ctx.enter_context(tc.tile_pool(name="dram", bufs=1, space="DRAM"))

        stat = [
            statp.tile([128, KT, G], F16, name=f"stat{i}", tag=f"stat{i}")
            for i in range(2)
        ]
        slab = [
            slabs.tile([128, KT, SW], F16, name=f"slab{i}", tag=f"slab{i}")
            for i in range(3)
        ]
        xts = slabs.tile([128, KT, BW], F16, name="xts", tag="xts")

        q = [nc.sync, nc.gpsimd]  # SP HWDGE + software DGE

        # prologue: first slab, then W rows in k order (k-major s1 MMs
        # start consuming after the first row lands)
        for k in range(KT):
            q[k % 2].dma_start(slab[0][:, k, :], u1[128 * k : 128 * (k + 1), :])
        for k in range(KT):
            q[k % 2].dma_start(stat[0][:, k, :], w16[128 * k : 128 * (k + 1), :])

        def half_mms(st, rhs, out, scale, h, kmajor):
            mr = range(8 * h, 8 * h + 8)
            pss = {
                m: mmps.tile([128, BW], F32, name="ps", tag="ps")[:, 0:SW]
                for m in mr
            }
            if kmajor:
                for k in range(KT):
                    for m in mr:
                        nc.tensor.matmul(
                            pss[m],
                            st[:, k, 128 * m : 128 * (m + 1)],
                            rhs[:, k, :],
                            start=(k == 0),
                            stop=(k == KT - 1),
                        )
            else:
                for m in mr:
                    for k in range(KT):
                        nc.tensor.matmul(
                            pss[m],
                            st[:, k, 128 * m : 128 * (m + 1)],
                            rhs[:, k, :],
                            start=(k == 0),
                            stop=(k == KT - 1),
                        )
            for m in mr:
                nc.vector.tensor_scalar_mul(out[:, m, :], pss[m], scale)

        for si, (pw, sb, shift, export, kmajor) in enumerate(_STEPS):
            st = stat[sb]
            rhs = slab[si % 3]
            out = slab[(si + 1) % 3]
            scale = float(2.0**shift)
            for h in range(2):
                half_mms(st, rhs, out, scale, h, kmajor and h == 0)
            if export:
                # single full-slab AllGather -> next stationary via XBAR
                u_in = dram.tile([G, SW], F16, name=f"uin{pw}", tag=f"uin{pw}")
                for k in range(KT):
                    q[k % 2].dma_start(
                        u_in[128 * k : 128 * (k + 1), :], out[:, k, :]
                    )
                g = dram.tile(
                    [_NC * G, SW],
                    F16,
                    name=f"g{pw}",
                    tag=f"g{pw}",
                    addr_space="Shared",
                )
                nc.gpsimd.collective_compute(
                    "AllGather",
                    mybir.AluOpType.bypass,
                    replica_groups=rg,
                    ins=[u_in.opt()],
                    outs=[g.opt()],
                )
                # stat[:, k, :] = T[128k+p, :] — one XBAR row per k-tile
                nst = stat[1 - sb]
                for k in range(KT):
                    j = k // 2
                    c0 = 128 * (k % 2)
                    nc.sync.dma_start(
                        nst[:, k, :],
                        g[G * j : G * (j + 1), c0 : c0 + 128],
                        transpose=True,
                    )
            if si == 1:
                # stream x.T during the long T6/T9/T12 burst
                for k in range(KT):
                    q[k % 2].dma_start(
                        xts[:, k, :], xt[128 * k : 128 * (k + 1), :]
                    )

        # final raw-slab AllGather: g60 blocks are directly A60-scaled
        u60 = slab[len(_STEPS) % 3]
        u60_in = dram.tile([G, SW], F16, name="u60in", tag="u60in")
        for k in range(KT):
            q[k % 2].dma_start(u60_in[128 * k : 128 * (k + 1), :], u60[:, k, :])
        g60 = dram.tile(
            [G * _NC, SW], F16, name="g60", tag="g60", addr_space="Shared"
        )
        nc.gpsimd.collective_compute(
            "AllGather",
            mybir.AluOpType.bypass,
            replica_groups=rg,
            ins=[u60_in.opt()],
            outs=[g60.opt()],
        )

        # apply: ytj = (x_j @ A60s).T ; A60s[f, f'] = g60[G*(f'//SW) + f, f'%SW]
        # j-block-major: each block's 16 chunks feed the 2 m-tiles on it
        ast = stat[1]
        for j in range(_NC):
            for k in range(KT):
                q[k % 2].dma_start(
                    ast[:, k, SW * j : SW * (j + 1)],
                    g60[G * j + 128 * k : G * j + 128 * (k + 1), :],
                )
            for m in (2 * j, 2 * j + 1):
                psY = mmps.tile([128, BW], F32, name="ps", tag="ps")
                for k in range(KT):
                    nc.tensor.matmul(
                        psY[:],
                        ast[:, k, 128 * m : 128 * (m + 1)],
                        xts[:, k, :],
                        start=(k == 0),
                        stop=(k == KT - 1),
                    )
                ystage = ypool.tile([128, BW], F32, name="ystage", tag="ystage")
                nc.vector.tensor_copy(ystage[:], psY[:])
                q[m % 2].dma_start(ytj[128 * m : 128 * (m + 1), :], ystage[:])
    nc.compile()
    return nc


def kernel(x, W):
    from concourse.bass_utils import run_bass_kernel_spmd

    if "nc" not in _cache:
        _cache["nc"] = _build()
    nc = _cache["nc"]

    W = np.asarray(W, dtype=np.float32)
    x = np.asarray(x, dtype=np.float32)
    w16_np = np.ascontiguousarray(W.astype(np.float16))
    xt_np = x.T.astype(np.float16)
    in_maps = [
        {
            "w16": w16_np,
            "u1": np.ascontiguousarray(w16_np[_SW * j : _SW * (j + 1), :].T),
            "xt": np.ascontiguousarray(xt_np[:, _BW * j : _BW * (j + 1)]),
        }
        for j in range(_NC)
    ]
    res = run_bass_kernel_spmd(nc, in_maps, core_ids=list(range(_NC)))
    _cache["last_exec_time_ns"] = res.exec_time_ns
    _cache["last_results"] = res
    unscale = np.float32(2.0**-_TOTAL_SHIFT)
    y = np.concatenate(
        [res.results[j]["ytj"].T * unscale for j in range(_NC)], axis=0
    ).astype(np.float32)
    return y


# revision 14
# speedup vs baseline: 1.7554x; 1.0385x over previous
"""Trainium2 kernel for nn_IteratedLinearNet: y = x @ (W.T)^60.

Strategy (8 NeuronCores, single SPMD launch):
  - track T_k = W^k; 60 = 12 + 48, so  y = x @ A12 @ A48  with
    A_k = (W.T)^k = T_k.T — a TWO-STAGE apply whose stationaries come
    straight from raw slab AllGathers (no transpose needed)
  - chain: 8 products using only THREE stationary matrices {W, T3, T12}:
      T2=T1*W, T3=T2*W                       (stationary: W, DRAM input)
      T6=T3*T3, T9=T6*T3, T12=T9*T3          (stationary: gathered T3)
      T24=T12*T12, T36=T24*T12, T48=T36*T12  (stationary: gathered T12)
  - 3 AllGathers total: T3 and T12 land in the chain stationary via
    XBAR-transpose (T12's raw blocks also feed the stage-1 apply
    stationary A12 directly), T48 raw feeds the stage-2 stationary A48;
    the stage-1 apply (z = x @ A12) overlaps the T24..T48 stationary load
  - a tiny warmup AllGather runs during the prologue to absorb the
    collective first-use penalty (~45us otherwise on the first gather)
  - steps that follow a stationary load run their matmuls K-MAJOR with
    8 open PSUM groups so MM consumption tracks stationary-row arrival
  - plain DMAs alternate between the SP HWDGE queue and the gpsimd
    software DGE (the Activation HWDGE queue corrupts data on this
    runtime); XBAR transposes are HWDGE-only so they stay on SP
  - float16 at full PE rate with per-step power-of-2 rescaling (exact,
    folded into PSUM evacuation); PSUM accumulates fp32; host multiplies
    the fp32 output by 2^-50.  Simulated end-to-end rel err ~2.3e-3.

Self-contained: builds/compiles on first call and caches the module.
"""

import numpy as np

_G = 2048
_B = 4096
_NC = 8
_SW = _G // _NC  # 256 slab width (rows per core)
_BW = _B // _NC  # 512 batch rows per core
_KT = _G // 128  # 16

# (power, stat: 0=W/g12(stat0) 1=g3(stat1), shift, export, kmajor)
# slab ring: step i reads slab[i%3], writes slab[(i+1)%3]
_STEPS = [
    (2, 0, 6, False, True),
    (3, 0, 1, True, False),  # -> g3 (XBAR -> stat1)
    (6, 1, -5, False, True),
    (9, 1, -4, False, False),
    (12, 1, -5, True, False),  # -> g12 (XBAR -> stat0, raw -> A12/stat1)
    (24, 0, -5, False, True),
    (36, 0, -4, False, False),
    (48, 0, -5, True, False),  # -> g48 (raw -> A48/stat0)
]
_ZSHIFT = -6
_TOTAL_SHIFT = 50  # sh12(14) + sh48(42) + zshift(-6)

_cache = {}


def _build():
    from contextlib import ExitStack

    import concourse.tile as tile
    from concourse import bacc, mybir

    F16 = mybir.dt.float16
    F32 = mybir.dt.float32
    G, KT, SW, BW = _G, _KT, _SW, _BW

    nc = bacc.Bacc(None, target_bir_lowering=False, num_devices=_NC)
    w16 = nc.declare_dram_parameter("w16", [G, G], F16, isOutput=False)
    u1 = nc.declare_dram_parameter("u1", [G, SW], F16, isOutput=False)
    xt = nc.declare_dram_parameter("xt", [G, BW], F16, isOutput=False)
    ytj = nc.declare_dram_parameter("ytj", [G, BW], F32, isOutput=True)

    rg = [list(range(_NC))]

    with ExitStack() as ctx:
        tc = ctx.enter_context(tile.TileContext(nc))
        statp = ctx.enter_context(tc.tile_pool(name="statp", bufs=1))
        slabs = ctx.enter_context(tc.tile_pool(name="slabs", bufs=1))
        ypool = ctx.enter_context(tc.tile_pool(name="ypool", bufs=2))
        mmps = ctx.enter_context(tc.tile_pool(name="mmps", bufs=8, space="PSUM"))
        dram = ctx.enter_context(tc.tile_pool(name="dram", bufs=1, space="DRAM"))

        stat = [
            statp.tile([128, KT, G], F16, name=f"stat{i}", tag=f"stat{i}")
            for i in range(2)
        ]
        slab = [
            slabs.tile([128, KT, SW], F16, name=f"slab{i}", tag=f"slab{i}")
            for i in range(3)
        ]
        xts = slabs.tile([128, KT, BW], F16, name="xts", tag="xts")
        zt = slabs.tile([128, KT, BW], F16, name="zt", tag="zt")

        q = [nc.sync, nc.gpsimd]  # SP HWDGE + software DGE

        # warmup AllGather: absorbs the collective first-use penalty
        # while the prologue DMAs run; output is never consumed
        warm_in = dram.tile([128, 64], F16, name="warm_in", tag="warm_in")
        nc.sync.dma_start(warm_in[:, :], u1[0:128, 0:64])
        warm_out = dram.tile(
            [128 * _NC, 64], F16, name="warm_out", tag="warm_out",
            addr_space="Shared",
        )
        nc.gpsimd.collective_compute(
            "AllGather",
            mybir.AluOpType.bypass,
            replica_groups=rg,
            ins=[warm_in.opt()],
            outs=[warm_out.opt()],
        )

        # prologue: first slab, then W rows in k order (k-major s1 MMs
        # start consuming after the first row lands)
        for k in range(KT):
            q[k % 2].dma_start(slab[0][:, k, :], u1[128 * k : 128 * (k + 1), :])
        for k in range(KT):
            q[k % 2].dma_start(stat[0][:, k, :], w16[128 * k : 128 * (k + 1), :])

        def half_mms(st, rhs, out, scale, h, kmajor):
            mr = range(8 * h, 8 * h + 8)
            pss = {
                m: mmps.tile([128, BW], F32, name="ps", tag="ps")[:, 0:SW]
                for m in mr
            }
            order = (
                [(k, m) for k in range(KT) for m in mr]
                if kmajor
                else [(k, m) for m in mr for k in range(KT)]
            )
            for k, m in order:
                nc.tensor.matmul(
                    pss[m],
                    st[:, k, 128 * m : 128 * (m + 1)],
                    rhs[:, k, :],
                    start=(k == 0),
                    stop=(k == KT - 1),
                )
            for m in mr:
                nc.vector.tensor_scalar_mul(out[:, m, :], pss[m], scale)

        def emit_apply(ast, g, rhs_t, out_evac):
            """ast[:, k, SW*j:...] <- g blocks, j-major; 2 MM tiles per block.
            out_evac(m, psY) consumes the [128, BW] fp32 PSUM tile."""
            for j in range(_NC):
                for k in range(KT):
                    q[k % 2].dma_start(
                        ast[:, k, SW * j : SW * (j + 1)],
                        g[G * j + 128 * k : G * j + 128 * (k + 1), :],
                    )
            for m in range(KT):
                psY = mmps.tile([128, BW], F32, name="ps", tag="ps")
                for k in range(KT):
                    nc.tensor.matmul(
                        psY[:],
                        ast[:, k, 128 * m : 128 * (m + 1)],
                        rhs_t[:, k, :],
                        start=(k == 0),
                        stop=(k == KT - 1),
                    )
                out_evac(m, psY)

        ag_raw = {}
        for si, (pw, sb, shift, export, kmajor) in enumerate(_STEPS):
            st = stat[sb]
            rhs = slab[si % 3]
            out = slab[(si + 1) % 3]
            scale = float(2.0**shift)
            for h in range(2):
                half_mms(st, rhs, out, scale, h, kmajor and h == 0)
            if export:
                u_in = dram.tile([G, SW], F16, name=f"uin{pw}", tag=f"uin{pw}")
                for k in range(KT):
                    q[k % 2].dma_start(
                        u_in[128 * k : 128 * (k + 1), :], out[:, k, :]
                    )
                g = dram.tile(
                    [_NC * G, SW], F16, name=f"g{pw}", tag=f"g{pw}",
                    addr_space="Shared",
                )
                nc.gpsimd.collective_compute(
                    "AllGather",
                    mybir.AluOpType.bypass,
                    replica_groups=rg,
                    ins=[u_in.opt()],
                    outs=[g.opt()],
                )
                ag_raw[pw] = g
                if pw == 12:
                    # stage-1 apply first: its A12 chunk DMAs must lead
                    # the XBAR rows on the SP queue so z = x @ A12 starts
                    # right after the gather; the T24..T48 stationary
                    # loads behind it while the PE chews on the apply
                    zscale = float(2.0**_ZSHIFT)
                    emit_apply(
                        stat[1],  # g3 is dead after T12; reuse for A12
                        g,
                        xts,
                        lambda m, psY: nc.vector.tensor_scalar_mul(
                            zt[:, m, :], psY[:], zscale
                        ),
                    )
                if pw != 48:
                    # XBAR-transpose gathered raw slabs into the chain
                    # stationary: stat[:, k, :] = T[128k+p, :]
                    nst = stat[1 - sb]
                    for k in range(KT):
                        j = k // 2
                        c0 = 128 * (k % 2)
                        nc.sync.dma_start(
                            nst[:, k, :],
                            g[G * j : G * (j + 1), c0 : c0 + 128],
                            transpose=True,
                        )
            if si == 1:
                # stream x.T during the long T6/T9/T12 burst
                for k in range(KT):
                    q[k % 2].dma_start(
                        xts[:, k, :], xt[128 * k : 128 * (k + 1), :]
                    )

        # stage-2 apply: y = z @ A48 (A48 blocks raw from g48)
        def y_evac(m, psY):
            ystage = ypool.tile([128, BW], F32, name="ystage", tag="ystage")
            nc.vector.tensor_copy(ystage[:], psY[:])
            q[m % 2].dma_start(ytj[128 * m : 128 * (m + 1), :], ystage[:])

        emit_apply(stat[0], ag_raw[48], zt, y_evac)
    nc.compile()
    return nc


def kernel(x, W):
    from concourse.bass_utils import run_bass_kernel_spmd

    if "nc" not in _cache:
        _cache["nc"] = _build()
    nc = _cache["nc"]

    W = np.asarray(W, dtype=np.float32)
    x = np.asarray(x, dtype=np.float32)
    w16_np = np.ascontiguousarray(W.astype(np.float16))
    xt_np = x.T.astype(np.float16)
    in_maps = [
        {
            "w16": w16_np,
            "u1": np.ascontiguousarray(w16_np[_SW * j : _SW * (j + 1), :].T),
            "xt": np.ascontiguousarray(xt_np[:, _BW * j : _BW * (j + 1)]),
        }
        for j in range(_NC)
    ]
    res = run_bass_kernel_spmd(nc, in_maps, core_ids=list(range(_NC)))
    _cache["last_exec_time_ns"] = res.exec_time_ns
    _cache["last_results"] = res
    unscale = np.float32(2.0**-_TOTAL_SHIFT)
    y = np.concatenate(
        [res.results[j]["ytj"].T * unscale for j in range(_NC)], axis=0
    ).astype(np.float32)
    return y
